# revision 1
# baseline (speedup 1.0000x reference)
"""CPMAnt attention kernel for 8 TRN2 NeuronCores.

Sharding: tensor-parallel over heads. Each core computes 4 of the 32 heads:
  q/k/v projections with column-sliced Wq/Wk/Wv, attention with its slice of
  position_bias, and a partial output projection with the row-sliced Wo.
The 8 partial outputs [B,S,D] are summed on the host (the all-reduce).

Device layout trick: the host pre-transposes hidden to hT = hidden^T [D, B*S]
so every matmul on the device uses natural (non-transposed) operand loads:
  qT/kT [dh, rows] = Wx^T-slice @ hidden^T   (lhsT = Wx tiles, rhs = hT tiles)
  v     [rows, dh] = hidden @ Wv-slice       (lhsT = hT tiles, rhs = Wv tiles)
  scores[q, k]     = qh^T.T @ kh^T
  probsT[k, q]     = PE-transpose of softmax(scores), fused with the
                     1/rowsum normalization by transposing against
                     diag(1/rowsum) instead of the identity
  ctxT  [dh, q]    = v-tiles.T @ probsT
  outT  [D, rows]  = Wo-slice tiles.T @ ctxT        (partial, summed on host)

softmax is computed without max-subtraction: scores = q.k/sqrt(128)+bias are
bounded (|.| < ~20 for this problem's N(0,1) data), far from fp32 exp
overflow, and masked positions are -30000 so exp underflows to exactly 0,
which also reproduces the reference's post-softmax mask zeroing.

Precision: fp32r (TF32-like, full PE rate at free-dim>=512) for the q/k
projections and scores; fp16 (also 10 mantissa bits) for probs/v/ctx/Wo.
"""

import math

import numpy as np

B, S, D = 2, 1024, 4096
H, DH = 32, 128
NCORES = 8
HPC = H // NCORES  # heads per core = 4
R = B * S  # 2048 rows
KT = D // 128  # 32 contraction tiles for the projections
NB = R // 512  # 4 row blocks
SCALE = 1.0 / math.sqrt(DH)
MASK_NEG = -30000.0


def _build_core_kernel(repeat: int = 1):
    import concourse.mybir as mybir
    from concourse import bacc
    from concourse.tile import TileContext
    from concourse.masks import make_identity

    f32 = mybir.dt.float32
    f32r = mybir.dt.float32r
    bf16 = mybir.dt.bfloat16
    fp16 = mybir.dt.float16
    Exp = mybir.ActivationFunctionType.Exp

    nc = bacc.Bacc("TRN2")

    hqT = nc.declare_dram_parameter("hqT", [D, R], fp16, isOutput=False)
    hkvT = nc.declare_dram_parameter("hkvT", [D, R], fp16, isOutput=False)
    wq = nc.declare_dram_parameter("wq", [D, 512], fp16, isOutput=False)
    wk = nc.declare_dram_parameter("wk", [D, 512], fp16, isOutput=False)
    wv = nc.declare_dram_parameter("wv", [D, 512], fp16, isOutput=False)
    wo = nc.declare_dram_parameter("wo", [512, D], fp16, isOutput=False)
    bias = nc.declare_dram_parameter("bias", [B, HPC, S, S], fp16, isOutput=False)
    outT = nc.declare_dram_parameter("outT", [D, R], f32, isOutput=True)

    hq3 = hqT.rearrange("(t p) r -> p t r", p=128)  # [128, 32, 2048]
    hkv3 = hkvT.rearrange("(t p) r -> p t r", p=128)
    wq3 = wq.rearrange("(t p) m -> p t m", p=128)  # [128, 32, 512]
    wk3 = wk.rearrange("(t p) m -> p t m", p=128)
    wv3 = wv.rearrange("(t p) m -> p t m", p=128)
    wo3 = wo.rearrange("(t p) m -> p t m", p=128)  # [128, 4, 4096]
    outT3 = outT.rearrange("(m p) r -> p m r", p=128)  # [128, 32, 2048]

    with TileContext(nc) as tc:
      for _rep in range(repeat):
        with (
            tc.tile_pool(name="persist", bufs=1) as pers,
            tc.tile_pool(name="small", bufs=2) as spool,
        ):
            # Persistent SBUF tensors
            qT_s = pers.tile([128, HPC, R], fp16)  # 32KB/part
            kT_s = pers.tile([128, HPC, R], fp16)  # 32KB/part
            v_s = pers.tile([128, 16, 512], fp16)  # 16KB/part
            ctxT_s = pers.tile([128, HPC, R], fp16)  # 16KB/part
            ident = pers.tile([128, 128], fp16)
            make_identity(nc, ident)

            # q/k projections: xT[m, r] += W[kt, m].T @ hT[kt, r]
            def qk_proj(wpool, hpool, w3, hsrc3, dst, scale):
                with tc.tile_pool(name="ppsum", bufs=2, space="PSUM") as pp:
                    quarters = []
                    w_engines = [nc.sync, nc.scalar, nc.scalar, nc.scalar]
                    for qt in range(4):
                        wh = wpool.tile([128, 8, 512], fp16, tag="W", name="wh")
                        if qt == 0:
                            for sl in range(4):
                                nc.sync.dma_start(
                                    out=wh[:, sl * 2 : (sl + 1) * 2, :],
                                    in_=w3[:, sl * 2 : (sl + 1) * 2, :],
                                )
                        quarters.append(wh)
                    first_ht = hpool.tile([128, 4, 512], fp16, tag="ht", name="ht")
                    for kl in range(4):
                        nc.sync.dma_start(
                            out=first_ht[:, kl, :], in_=hsrc3[:, kl, 0:512]
                        )
                    for qt in range(1, 4):
                        w_engines[qt].dma_start(
                            out=quarters[qt], in_=w3[:, qt * 8 : (qt + 1) * 8, :]
                        )
                    for n in range(NB):
                        psums = [
                            pp.tile([128, 512], f32, tag=f"pp{m}", name=f"pp{m}")
                            for m in range(4)
                        ]
                        for ktg in range(KT // 4):
                            if n == 0 and ktg == 0:
                                ht = first_ht
                            else:
                                ht = hpool.tile([128, 4, 512], fp16, tag="ht", name="ht")
                                (nc.sync if ktg % 2 == 0 else nc.scalar).dma_start(
                                    out=ht,
                                    in_=hsrc3[:, ktg * 4 : (ktg + 1) * 4, n * 512 : (n + 1) * 512],
                                )
                            for kl in range(4):
                                kt = ktg * 4 + kl
                                wh = quarters[kt // 8]
                                for m in range(4):
                                    nc.tensor.matmul(
                                        psums[m],
                                        wh[:, kt % 8, m * 128 : (m + 1) * 128],
                                        ht[:, kl, :],
                                        start=(kt == 0),
                                        stop=(kt == KT - 1),
                                    )
                        for m in range(4):
                            nc.scalar.mul(
                                out=dst[:, m, n * 512 : (n + 1) * 512],
                                in_=psums[m],
                                mul=scale,
                            )

            # v projection: v[r, c] += hT[kt, r].T @ Wv[kt, c]
            def v_proj(wpool, hpool):
                with tc.tile_pool(name="vpsum", bufs=2, space="PSUM") as vp:
                    quarters = []
                    for qt in range(4):
                        wh = wpool.tile([128, 8, 512], fp16, tag="W", name="wh")
                        (nc.sync if qt % 2 == 0 else nc.scalar).dma_start(
                            out=wh, in_=wv3[:, qt * 8 : (qt + 1) * 8, :]
                        )
                        quarters.append(wh)
                    for rtg in range(4):  # groups of 4 row-tiles
                        psums = [
                            vp.tile([128, 512], f32, tag=f"vp{j}", name=f"vp{j}")
                            for j in range(4)
                        ]
                        for ktg in range(KT // 4):
                            ht = hpool.tile([128, 4, 512], fp16, tag="ht", name="ht")
                            (nc.sync if ktg % 2 == 0 else nc.scalar).dma_start(
                                out=ht,
                                in_=hkv3[:, ktg * 4 : (ktg + 1) * 4, rtg * 512 : (rtg + 1) * 512],
                            )
                            for kl in range(4):
                                kt = ktg * 4 + kl
                                wh = quarters[kt // 8]
                                for j in range(4):
                                    nc.tensor.matmul(
                                        psums[j],
                                        ht[:, kl, j * 128 : (j + 1) * 128],
                                        wh[:, kt % 8, :],
                                        start=(kt == 0),
                                        stop=(kt == KT - 1),
                                    )
                        for j in range(4):
                            nc.scalar.copy(out=v_s[:, rtg * 4 + j, :], in_=psums[j])

            with (
                tc.tile_pool(name="wpool", bufs=4) as wpool,
                tc.tile_pool(name="hstream", bufs=6) as hpool,
            ):
                qk_proj(wpool, hpool, wq3, hq3, qT_s, SCALE)
                qk_proj(wpool, hpool, wk3, hkv3, kT_s, 1.0)
                v_proj(wpool, hpool)

            # attention + output projection, per 512-row block
            with (
                tc.tile_pool(name="wopool", bufs=1) as wopool,
                tc.tile_pool(name="attn", bufs=3) as apool,
                tc.tile_pool(name="obuf", bufs=4) as opool,
                tc.tile_pool(name="spsum", bufs=2, space="PSUM") as sps,
                tc.tile_pool(name="tpsum", bufs=1, space="PSUM") as tps,
                tc.tile_pool(name="cpsum", bufs=1, space="PSUM") as cps,
                tc.tile_pool(name="opsum", bufs=2, space="PSUM") as ops,
            ):
                wo_s = wopool.tile([128, HPC, D], fp16)  # 32KB/part
                nc.scalar.dma_start(out=wo_s, in_=wo3)

                for n in range(NB):
                    b, qb = divmod(n, 2)
                    for h in range(HPC):
                        probsT = apool.tile(
                            [128, 8, 512], fp16, tag="probsT", name="probsT"
                        )
                        for qs in range(4):
                            q0 = n * 512 + qs * 128  # global row
                            qi = qb * 512 + qs * 128  # row within batch
                            s_ps = sps.tile([128, 1024], f32, tag="s", name="s_ps")
                            for kb in range(2):
                                nc.tensor.matmul(
                                    s_ps[:, kb * 512 : (kb + 1) * 512],
                                    qT_s[:, h, q0 : q0 + 128],
                                    kT_s[
                                        :,
                                        h,
                                        b * 1024 + kb * 512 : b * 1024 + (kb + 1) * 512,
                                    ],
                                    start=True,
                                    stop=True,
                                )
                            if qs % 2 == 0:
                                bias_t = apool.tile(
                                    [128, 2, 1024], fp16, tag="bias", name="bias_t"
                                )
                                nc.scalar.dma_start(
                                    out=bias_t,
                                    in_=bias[b, h].rearrange(
                                        "(s p) k -> p s k", p=128
                                    )[:, qb * 4 + qs : qb * 4 + qs + 2, :],
                                )
                            nc.vector.tensor_add(
                                out=s_ps, in0=s_ps, in1=bias_t[:, qs % 2, :]
                            )
                            probsU = apool.tile(
                                [128, 1024], fp16, tag="probsU", name="probsU"
                            )
                            rowsum = spool.tile(
                                [128, 1], f32, tag="rowsum", name="rowsum"
                            )
                            nc.scalar.activation(
                                out=probsU, in_=s_ps, func=Exp, accum_out=rowsum
                            )
                            recip = spool.tile([128, 1], f32, tag="recip", name="recip")
                            nc.vector.reciprocal(out=recip, in_=rowsum)
                            # PE transpose_mode ignores the identity operand's
                            # VALUES (pure transpose datapath), so the softmax
                            # normalization must happen before the transpose.
                            probsN = apool.tile(
                                [128, 1024], fp16, tag="probsN", name="probsN"
                            )
                            nc.vector.tensor_scalar_mul(
                                out=probsN, in0=probsU, scalar1=recip
                            )
                            for g in range(2):
                                t_ps = tps.tile([128, 512], fp16, tag="t", name="t_ps")
                                for j in range(4):
                                    kk = g * 4 + j
                                    nc.tensor.transpose(
                                        t_ps[:, j * 128 : (j + 1) * 128],
                                        probsN[:, kk * 128 : (kk + 1) * 128],
                                        ident,
                                    )
                                nc.vector.tensor_copy(
                                    out=probsT[
                                        :, g * 4 : (g + 1) * 4, qs * 128 : (qs + 1) * 128
                                    ],
                                    in_=t_ps.rearrange("p (j q) -> p j q", j=4),
                                )
                        c_ps = cps.tile([128, 512], f32, tag="c", name="c_ps")
                        for kt in range(8):
                            nc.tensor.matmul(
                                c_ps,
                                v_s[:, b * 8 + kt, h * 128 : (h + 1) * 128],
                                probsT[:, kt, :],
                                start=(kt == 0),
                                stop=(kt == 7),
                            )
                        nc.scalar.copy(
                            out=ctxT_s[:, h, n * 512 : (n + 1) * 512], in_=c_ps
                        )
                    # output projection for this row block
                    for m in range(KT):
                        o_ps = ops.tile([128, 512], f32, tag="o", name="o_ps")
                        for t in range(HPC):
                            nc.tensor.matmul(
                                o_ps,
                                wo_s[:, t, m * 128 : (m + 1) * 128],
                                ctxT_s[:, t, n * 512 : (n + 1) * 512],
                                start=(t == 0),
                                stop=(t == HPC - 1),
                            )
                        osb = opool.tile([128, 512], f32, tag="osb", name="osb")
                        if m % 2 == 0:
                            nc.scalar.copy(out=osb, in_=o_ps)
                        else:
                            nc.vector.tensor_copy(out=osb, in_=o_ps)
                        dmae = nc.sync if m % 2 == 0 else nc.gpsimd
                        dmae.dma_start(
                            out=outT3[:, m, n * 512 : (n + 1) * 512], in_=osb
                        )

    nc.compile()
    return nc


_NC_CACHE = None


def _round_tf32(a: np.ndarray) -> np.ndarray:
    """Round fp32 to tf32 (10 explicit mantissa bits), round-to-nearest-even.
    Matches the rounding the fp32r casting DMA performs, so it can be done
    once on the host and the device loads become plain HWDGE copies."""
    b = np.ascontiguousarray(a, dtype=np.float32).view(np.uint32)
    b = (b + np.uint32(0xFFF) + ((b >> np.uint32(13)) & np.uint32(1))) & np.uint32(
        0xFFFFE000
    )
    return b.view(np.float32)


def _prep_in_maps(
    hidden_q, hidden_kv, attention_mask, position_bias, Wq, Wk, Wv, Wo
):
    import ml_dtypes

    hqT = np.ascontiguousarray(
        np.asarray(hidden_q, dtype=np.float32).reshape(R, D).T
    ).astype(np.float16)
    hkvT = np.ascontiguousarray(
        np.asarray(hidden_kv, dtype=np.float32).reshape(R, D).T
    ).astype(np.float16)
    mask = np.asarray(attention_mask)
    pb = np.asarray(position_bias, dtype=np.float32)

    in_maps = []
    for c in range(NCORES):
        h0 = c * HPC
        bias_c = np.where(
            mask[:, None, :, :], pb[:, h0 : h0 + HPC], np.float32(MASK_NEG)
        ).astype(np.float32)
        in_maps.append(
            {
                "hqT": hqT,
                "hkvT": hkvT,
                "wq": np.ascontiguousarray(Wq[:, h0 * DH : (h0 + HPC) * DH]).astype(np.float16),
                "wk": np.ascontiguousarray(Wk[:, h0 * DH : (h0 + HPC) * DH]).astype(np.float16),
                "wv": np.ascontiguousarray(Wv[:, h0 * DH : (h0 + HPC) * DH]).astype(np.float16),
                "wo": np.ascontiguousarray(
                    Wo[h0 * DH : (h0 + HPC) * DH, :]
                ).astype(np.float16),
                "bias": bias_c.astype(np.float16),
            }
        )
    return in_maps


def kernel(
    hidden_q: np.ndarray,
    hidden_kv: np.ndarray,
    attention_mask: np.ndarray,
    position_bias: np.ndarray,
    Wq: np.ndarray,
    Wk: np.ndarray,
    Wv: np.ndarray,
    Wo: np.ndarray,
) -> np.ndarray:
    from concourse.bass_utils import run_bass_kernel_spmd

    global _NC_CACHE
    if _NC_CACHE is None:
        _NC_CACHE = _build_core_kernel()
    nc = _NC_CACHE

    in_maps = _prep_in_maps(
        hidden_q, hidden_kv, attention_mask, position_bias, Wq, Wk, Wv, Wo
    )
    res = run_bass_kernel_spmd(nc, in_maps, list(range(NCORES)))
    acc = res.results[0]["outT"].astype(np.float32)
    for c in range(1, NCORES):
        acc += res.results[c]["outT"]
    return np.ascontiguousarray(acc.T).reshape(B, S, D)



# revision 2
# speedup vs baseline: 7.9664x; 7.9664x over previous
"""CPMAnt attention kernel for 8 TRN2 NeuronCores — v2.

Sharding: tensor-parallel over heads. Each core computes 4 of the 32 heads and
a partial output projection; the 8 fp16 partials are summed on the host.

v2 design (vs v1):
- scores are computed TRANSPOSED ([k, q] layout) so softmax probabilities come
  out with k on partitions — exactly the layout the ctx matmul needs —
  eliminating all 512 PE probs-transposes and their DVE copies.
- the position bias (pre-masked, pre-transposed, fp16 on host) is injected
  into the scores PSUM by an identity matmul (start=True) that the k.q matmul
  accumulates onto (start=False), eliminating the DVE bias add.
- the softmax denominator comes free from a ones-column appended to V (ctx
  free dim 129), so ctx lands in [q, dh+1] with q on partitions where the
  1/rowsum normalization is a per-partition tensor_scalar; 4 small PE
  transposes per (block, head) produce ctxT for the output projection.
- fused k+v projection shares each streamed hkv tile (halves hkv traffic).
- k+v projection runs FIRST: its DMA demand (~130 GB/s) leaves slack to
  prefetch Wq/Wo/bias, so the q phase and attention never wait on DMA.
- output written fp16 (halves output traffic); host sums partials in f32.

softmax is computed without max-subtraction: scores are bounded (|.| < ~8)
and masked positions are -30000 so exp underflows to exactly 0, reproducing
the reference's post-softmax mask zeroing.
"""

import math

import numpy as np

B, S, D = 2, 1024, 4096
H, DH = 32, 128
NCORES = 8
HPC = H // NCORES  # heads per core = 4
R = B * S  # 2048 rows
KT = D // 128  # 32 contraction tiles for the projections
NB = R // 512  # 4 row blocks
SCALE = 1.0 / math.sqrt(DH)
MASK_NEG = -30000.0
VW = 132  # v_s row width: 128 dh + ones col + pad


def _build_core_kernel(repeat: int = 1):
    import concourse.mybir as mybir
    from concourse import bacc
    from concourse.tile import TileContext
    from concourse.masks import make_identity

    f32 = mybir.dt.float32
    fp16 = mybir.dt.float16
    Exp = mybir.ActivationFunctionType.Exp

    nc = bacc.Bacc("TRN2")

    hqT = nc.declare_dram_parameter("hqT", [D, R], fp16, isOutput=False)
    hkvT = nc.declare_dram_parameter("hkvT", [D, R], fp16, isOutput=False)
    wq = nc.declare_dram_parameter("wq", [D, 512], fp16, isOutput=False)
    wk = nc.declare_dram_parameter("wk", [D, 512], fp16, isOutput=False)
    wv = nc.declare_dram_parameter("wv", [D, 512], fp16, isOutput=False)
    wo = nc.declare_dram_parameter("wo", [512, D], fp16, isOutput=False)
    biasT = nc.declare_dram_parameter("biasT", [B, HPC, S, S], fp16, isOutput=False)
    outT = nc.declare_dram_parameter("outT", [D, R], fp16, isOutput=True)

    hq3 = hqT.rearrange("(t p) r -> p t r", p=128)  # [128, 32, 2048]
    hkv3 = hkvT.rearrange("(t p) r -> p t r", p=128)
    wq3 = wq.rearrange("(t p) m -> p t m", p=128)  # [128, 32, 512]
    wk3 = wk.rearrange("(t p) m -> p t m", p=128)
    wv3 = wv.rearrange("(t p) m -> p t m", p=128)
    wo3 = wo.rearrange("(t p) m -> p t m", p=128)  # [128, 4, 4096]
    biasT5 = biasT.rearrange("b h (t p) q -> b h p t q", p=128)  # [b,h,128,8,1024]
    outT3 = outT.rearrange("(m p) r -> p m r", p=128)  # [128, 32, 2048]

    with TileContext(nc) as tc:
      for _rep in range(repeat):
        with (
            tc.tile_pool(name="persist", bufs=1) as pers,
            tc.tile_pool(name="small", bufs=3) as spool,
            tc.tile_pool(name="wqpool", bufs=1) as wqp,
        ):
            qT_s = pers.tile([128, HPC, R], fp16)  # 16KB/part
            kT_s = pers.tile([128, HPC, R], fp16)  # 16KB/part
            v_s = pers.tile([128, 16, HPC, VW], fp16)  # 16.5KB/part
            ident = pers.tile([128, 128], fp16)
            make_identity(nc, ident)
            nc.vector.memset(v_s[:, :, :, 128:129], 1.0)  # softmax-denom ones

            # ---------------- fused k + v projection ----------------
            # kT[m, r] += Wk[kt, m].T @ hkvT[kt, r]   (4 psum banks)
            # v[r, c]  += hkvT[kt, r].T @ Wv[kt, c]   (4 psum banks)
            _hq_cm = tc.tile_pool(name="hq", bufs=8)
            hqp = _hq_cm.__enter__()
            hq_pre = {}
            with (
                tc.tile_pool(name="wkvpool", bufs=1) as wkvp,
                tc.tile_pool(name="hkv", bufs=8) as hkvp,
                tc.tile_pool(name="kvpsum", bufs=1, space="PSUM") as kvps,
            ):
                wk_chunks, wv_chunks = [], []
                for kt0, nkt, eng, which in (
                    (0, 2, nc.sync, "k"), (0, 2, nc.sync, "v"),
                    (2, 6, nc.gpsimd, "k"), (2, 6, nc.gpsimd, "v"),
                    (8, 8, nc.gpsimd, "k"), (8, 8, nc.gpsimd, "v"),
                    (16, 8, nc.gpsimd, "k"), (16, 8, nc.gpsimd, "v"),
                    (24, 8, nc.gpsimd, "k"), (24, 8, nc.gpsimd, "v"),
                ):
                    wsrc = wk3 if which == "k" else wv3
                    wt = wkvp.tile([128, nkt, 512], fp16,
                                   tag=f"w{which}{kt0}", name=f"w{which}{kt0}")
                    eng.dma_start(out=wt, in_=wsrc[:, kt0 : kt0 + nkt, :])
                    (wk_chunks if which == "k" else wv_chunks).append(
                        (wt, kt0, nkt))

                def w_at(chunks, kt):
                    for wt, kt0, nkt in chunks:
                        if kt0 <= kt < kt0 + nkt:
                            return wt[:, kt - kt0, :]
                    raise AssertionError

                # q weights prefetched behind the k/v chunks on gpsimd
                wq_chunks = []
                for kt0 in range(0, KT, 8):
                    wt = wqp.tile([128, 8, 512], fp16, tag=f"wq{kt0}",
                                  name=f"wq{kt0}")
                    nc.gpsimd.dma_start(out=wt, in_=wq3[:, kt0 : kt0 + 8, :])
                    wq_chunks.append((wt, kt0, 8))
                # first hq tiles for the q phase, also behind on gpsimd
                for ktg in range(6):
                    ht = hqp.tile([128, 2, 512], fp16, tag="ht", name="ht")
                    nc.gpsimd.dma_start(
                        out=ht, in_=hq3[:, ktg * 2 : ktg * 2 + 2, 0:512])
                    hq_pre[ktg] = ht

                for n in range(NB):
                    kpsums = [
                        kvps.tile([128, 512], f32, tag=f"kp{m}", name=f"kp{m}")
                        for m in range(4)
                    ]
                    vpsums = [
                        kvps.tile([128, 512], f32, tag=f"vp{j}", name=f"vp{j}")
                        for j in range(4)
                    ]
                    for ktg in range(16):
                        ht = hkvp.tile([128, 2, 512], fp16, tag="ht", name="ht")
                        (nc.scalar if ktg % 2 == 0 else nc.sync).dma_start(
                            out=ht,
                            in_=hkv3[:, ktg * 2 : ktg * 2 + 2,
                                     n * 512 : (n + 1) * 512],
                        )
                        for kl in range(2):
                            kt = ktg * 2 + kl
                            wkt = w_at(wk_chunks, kt)
                            wvt = w_at(wv_chunks, kt)
                            for m in range(4):
                                nc.tensor.matmul(
                                    kpsums[m],
                                    wkt[:, m * 128 : (m + 1) * 128],
                                    ht[:, kl, :],
                                    start=(kt == 0),
                                    stop=(kt == KT - 1),
                                )
                            for j in range(4):
                                nc.tensor.matmul(
                                    vpsums[j],
                                    ht[:, kl, j * 128 : (j + 1) * 128],
                                    wvt,
                                    start=(kt == 0),
                                    stop=(kt == KT - 1),
                                )
                    for m in range(4):
                        eng = nc.scalar if m % 2 == 0 else nc.vector
                        (eng.copy if m % 2 == 0 else eng.tensor_copy)(
                            out=kT_s[:, m, n * 512 : (n + 1) * 512], in_=kpsums[m]
                        )
                    for j in range(4):
                        eng = nc.scalar if j % 2 == 0 else nc.vector
                        (eng.copy if j % 2 == 0 else eng.tensor_copy)(
                            out=v_s[:, n * 4 + j, :, 0:128],
                            in_=vpsums[j].rearrange("p (h c) -> p h c", h=HPC),
                        )

            # ---------------- q projection + attention scope ----------------
            _wop_cm = tc.tile_pool(name="wopool", bufs=1)
            _bp_cm = tc.tile_pool(name="bias", bufs=12)
            wop = _wop_cm.__enter__()
            bpool = _bp_cm.__enter__()
            bias_tiles = {}

            def prefetch_bias(n, h, eng):
                b, qb = divmod(n, 2)
                tiles = []
                for g in range(4):
                    bias_t = bpool.tile([128, 2, 512], fp16, tag="bias",
                                        name="bias_t")
                    eng.dma_start(
                        out=bias_t,
                        in_=biasT5[b, h][:, g * 2 : (g + 1) * 2,
                                         qb * 512 : (qb + 1) * 512],
                    )
                    tiles.append(bias_t)
                bias_tiles[(n, h)] = tiles

            # qT[m, r] += Wq[kt, m].T @ hqT[kt, r]; SCALE folded into Wq on host
            with (
                tc.tile_pool(name="qpsum", bufs=2, space="PSUM") as qps,
            ):
                # warm the Exp activation table (~2.7us) well before attention;
                # doing this at t=0 would block the scalar DMA ring.
                dummy = spool.tile([128, 1], f32, tag="dummy", name="dummy")
                nc.vector.memset(dummy, 0.0)
                nc.scalar.activation(out=dummy, in_=dummy, func=Exp)
                # prefetches for the attention phase (bandwidth is slack here)
                prefetch_bias(0, 0, nc.gpsimd)
                prefetch_bias(0, 1, nc.gpsimd)
                wo_s = wop.tile([128, HPC, D], fp16)  # 32KB/part
                nc.gpsimd.dma_start(out=wo_s, in_=wo3)

                def wq_at(kt):
                    for wt, kt0, nkt in wq_chunks:
                        if kt0 <= kt < kt0 + nkt:
                            return wt[:, kt - kt0, :]
                    raise AssertionError

                for n in range(NB):
                    psums = [
                        qps.tile([128, 512], f32, tag=f"qp{m}", name=f"qp{m}")
                        for m in range(4)
                    ]
                    for ktg in range(16):  # groups of 2 kt
                        if n == 0 and ktg < 6:
                            ht = hq_pre[ktg]
                        else:
                            ht = hqp.tile([128, 2, 512], fp16, tag="ht",
                                          name="ht")
                            (nc.scalar if ktg % 2 == 0 else nc.sync).dma_start(
                                out=ht,
                                in_=hq3[:, ktg * 2 : ktg * 2 + 2,
                                        n * 512 : (n + 1) * 512],
                            )
                        for kl in range(2):
                            kt = ktg * 2 + kl
                            for m in range(4):
                                nc.tensor.matmul(
                                    psums[m],
                                    wq_at(kt)[:, m * 128 : (m + 1) * 128],
                                    ht[:, kl, :],
                                    start=(kt == 0),
                                    stop=(kt == KT - 1),
                                )
                    for m in range(4):
                        eng = nc.scalar if m % 2 == 0 else nc.vector
                        (eng.copy if m % 2 == 0 else eng.tensor_copy)(
                            out=qT_s[:, m, n * 512 : (n + 1) * 512], in_=psums[m]
                        )

            # ---------------- attention + output projection ----------------
            with (
                tc.tile_pool(name="probs", bufs=2) as ppool,
                tc.tile_pool(name="ctxT", bufs=2) as ctpool,
                tc.tile_pool(name="obuf", bufs=4) as opool,
                tc.tile_pool(name="csb", bufs=4) as csbp,
                tc.tile_pool(name="spsum", bufs=2, space="PSUM") as sps,
                tc.tile_pool(name="cpsum", bufs=1, space="PSUM") as cps,
                tc.tile_pool(name="tpsum", bufs=1, space="PSUM") as tps,
                tc.tile_pool(name="opsum", bufs=2, space="PSUM") as ops,
            ):
                probs_of = {}
                ctxT_of = {}
                hseq = [(n, h) for n in range(NB) for h in range(HPC)]

                def scores(n, h):
                    """scoresT[k, q] = bias + K.Q^T; exp -> probsU [k, 8, 512]."""
                    b, qb = divmod(n, 2)
                    # prefetch bias two heads ahead on the now-idle sync queue
                    i = hseq.index((n, h))
                    if i + 2 < len(hseq):
                        prefetch_bias(*hseq[i + 2], nc.sync)
                    probsU = ppool.tile([128, 8, 512], fp16, tag="probs",
                                        name="probsU")
                    probs_of[(n, h)] = probsU
                    btiles = bias_tiles.pop((n, h))
                    for g in range(4):  # 2 k-tiles per group
                        s_ps = sps.tile([128, 2, 512], f32, tag="s", name="s_ps")
                        bias_t = btiles[g]
                        pe_bias = True  # DVE-add variant stalls the exp chain
                        for j in range(2):
                            kt = g * 2 + j
                            if pe_bias:
                                nc.tensor.matmul(
                                    s_ps[:, j, :], ident, bias_t[:, j, :],
                                    start=True, stop=False,
                                )
                            nc.tensor.matmul(
                                s_ps[:, j, :],
                                kT_s[:, h, b * 1024 + kt * 128 : b * 1024 + (kt + 1) * 128],
                                qT_s[:, h, n * 512 : (n + 1) * 512],
                                start=not pe_bias, stop=True,
                            )
                        if not pe_bias:
                            nc.vector.tensor_add(out=s_ps, in0=s_ps, in1=bias_t)
                        nc.scalar.activation(
                            out=probsU[:, g * 2 : (g + 1) * 2, :], in_=s_ps,
                            func=Exp,
                        )

                def ctx(n, h):
                    """ctx[q, dh+1] = probsU.T @ [v | 1]; normalize; transpose."""
                    b = n // 2
                    probsU = probs_of.pop((n, h))
                    t_ps = tps.tile([128, 4, 128], fp16, tag="t", name="t_ps")
                    for pair in range(2):
                        c_ps = cps.tile([128, 2, 130], f32, tag="c", name="c_ps")
                        for qt2 in range(2):
                            qt = pair * 2 + qt2
                            for kt in range(8):
                                nc.tensor.matmul(
                                    c_ps[:, qt2, 0:129],
                                    probsU[:, kt, qt * 128 : (qt + 1) * 128],
                                    v_s[:, b * 8 + kt, h, 0:129],
                                    start=(kt == 0),
                                    stop=(kt == 7),
                                )
                        for qt2 in range(2):
                            qt = pair * 2 + qt2
                            recip = spool.tile([128, 1], f32, tag="recip",
                                               name="recip")
                            nc.vector.reciprocal(out=recip,
                                                 in_=c_ps[:, qt2, 128:129])
                            ctx_sb = csbp.tile([128, 128], fp16, tag="ctxsb",
                                               name="ctx_sb")
                            nc.vector.tensor_scalar_mul(
                                out=ctx_sb, in0=c_ps[:, qt2, 0:128],
                                scalar1=recip,
                            )
                            nc.tensor.transpose(t_ps[:, qt, :], ctx_sb, ident)
                    ctxT = ctxT_of[n]
                    nc.vector.tensor_copy(
                        out=ctxT[:, h, :],
                        in_=t_ps.rearrange("p a q -> p (a q)"),
                    )

                def outproj(n):
                    ctxT = ctxT_of.pop(n)
                    for m in range(KT):
                        o_ps = ops.tile([128, 512], f32, tag="o", name="o_ps")
                        for t in range(HPC):
                            nc.tensor.matmul(
                                o_ps,
                                wo_s[:, t, m * 128 : (m + 1) * 128],
                                ctxT[:, t, :],
                                start=(t == 0),
                                stop=(t == HPC - 1),
                            )
                        osb = opool.tile([128, 512], fp16, tag="osb", name="osb")
                        if m % 2 == 0:
                            nc.scalar.copy(out=osb, in_=o_ps)
                        else:
                            nc.vector.tensor_copy(out=osb, in_=o_ps)
                        if n == NB - 1:
                            dmae = (nc.gpsimd, nc.scalar, nc.sync)[m % 3]
                        else:
                            dmae = (nc.gpsimd, nc.scalar)[m % 2]
                        dmae.dma_start(
                            out=outT3[:, m, n * 512 : (n + 1) * 512], in_=osb
                        )

                # software-pipelined emission: scores one head ahead of ctx;
                # scores(n+1, 0) before outproj(n) so exp overlaps the
                # output projection.
                for n in range(NB):
                    ctxT_of[n] = ctpool.tile([128, HPC, 512], fp16, tag="ctxT",
                                             name="ctxT")
                    if n == 0:
                        scores(0, 0)
                    for h in range(HPC):
                        if h < HPC - 1:
                            scores(n, h + 1)
                        ctx(n, h)
                    if n < NB - 1:
                        scores(n + 1, 0)
                    outproj(n)

            _bp_cm.__exit__(None, None, None)
            _wop_cm.__exit__(None, None, None)
            _hq_cm.__exit__(None, None, None)

    nc.compile()
    return nc


_NC_CACHE = None


def _prep_in_maps(
    hidden_q, hidden_kv, attention_mask, position_bias, Wq, Wk, Wv, Wo
):
    hqT = np.ascontiguousarray(
        np.asarray(hidden_q, dtype=np.float32).reshape(R, D).T
    ).astype(np.float16)
    hkvT = np.ascontiguousarray(
        np.asarray(hidden_kv, dtype=np.float32).reshape(R, D).T
    ).astype(np.float16)
    mask = np.asarray(attention_mask)  # [B, S, S] (q, k)
    pb = np.asarray(position_bias, dtype=np.float32)
    Wq = np.asarray(Wq, dtype=np.float32) * np.float32(SCALE)

    in_maps = []
    for c in range(NCORES):
        h0 = c * HPC
        bias_c = np.where(
            mask[:, None, :, :], pb[:, h0 : h0 + HPC], np.float32(MASK_NEG)
        )
        # transpose to [b, h, k, q]
        biasT_c = np.ascontiguousarray(bias_c.transpose(0, 1, 3, 2)).astype(
            np.float16
        )
        in_maps.append(
            {
                "hqT": hqT,
                "hkvT": hkvT,
                "wq": np.ascontiguousarray(
                    Wq[:, h0 * DH : (h0 + HPC) * DH]
                ).astype(np.float16),
                "wk": np.ascontiguousarray(
                    Wk[:, h0 * DH : (h0 + HPC) * DH]
                ).astype(np.float16),
                "wv": np.ascontiguousarray(
                    Wv[:, h0 * DH : (h0 + HPC) * DH]
                ).astype(np.float16),
                "wo": np.ascontiguousarray(
                    Wo[h0 * DH : (h0 + HPC) * DH, :]
                ).astype(np.float16),
                "biasT": biasT_c,
            }
        )
    return in_maps


def kernel(
    hidden_q: np.ndarray,
    hidden_kv: np.ndarray,
    attention_mask: np.ndarray,
    position_bias: np.ndarray,
    Wq: np.ndarray,
    Wk: np.ndarray,
    Wv: np.ndarray,
    Wo: np.ndarray,
) -> np.ndarray:
    from concourse.bass_utils import run_bass_kernel_spmd

    global _NC_CACHE
    if _NC_CACHE is None:
        _NC_CACHE = _build_core_kernel()
    nc = _NC_CACHE

    in_maps = _prep_in_maps(
        hidden_q, hidden_kv, attention_mask, position_bias, Wq, Wk, Wv, Wo
    )
    res = run_bass_kernel_spmd(nc, in_maps, list(range(NCORES)))
    acc = res.results[0]["outT"].astype(np.float32)
    for c in range(1, NCORES):
        acc += res.results[c]["outT"]
    return np.ascontiguousarray(acc.T).reshape(B, S, D)


# revision 4
# speedup vs baseline: 52.1473x; 6.5459x over previous
"""CPMAnt attention kernel for 8 TRN2 NeuronCores — v2.

Sharding: tensor-parallel over heads. Each core computes 4 of the 32 heads and
a partial output projection; the 8 fp16 partials are summed on the host.

v2 design (vs v1):
- scores are computed TRANSPOSED ([k, q] layout) so softmax probabilities come
  out with k on partitions — exactly the layout the ctx matmul needs —
  eliminating all 512 PE probs-transposes and their DVE copies.
- the position bias (pre-masked, pre-transposed, fp16 on host) is injected
  into the scores PSUM by an identity matmul (start=True) that the k.q matmul
  accumulates onto (start=False), eliminating the DVE bias add.
- the softmax denominator comes free from a ones-column appended to V (ctx
  free dim 129), so ctx lands in [q, dh+1] with q on partitions where the
  1/rowsum normalization is a per-partition tensor_scalar; 4 small PE
  transposes per (block, head) produce ctxT for the output projection.
- fused k+v projection shares each streamed hkv tile (halves hkv traffic).
- k+v projection runs FIRST: its DMA demand (~130 GB/s) leaves slack to
  prefetch Wq/Wo/bias, so the q phase and attention never wait on DMA.
- output written fp16 (halves output traffic); host sums partials in f32.

softmax is computed without max-subtraction: scores are bounded (|.| < ~8)
and masked positions are -30000 so exp underflows to exactly 0, reproducing
the reference's post-softmax mask zeroing.
"""

import math

import numpy as np

B, S, D = 2, 1024, 4096
H, DH = 32, 128
NCORES = 8
HPC = H // NCORES  # heads per core = 4
R = B * S  # 2048 rows
KT = D // 128  # 32 contraction tiles for the projections
NB = R // 512  # 4 row blocks
SCALE = 1.0 / math.sqrt(DH)
MASK_NEG = -30000.0
VW = 132  # v_s row width: 128 dh + ones col + pad


def _build_core_kernel(repeat: int = 1):
    import concourse.mybir as mybir
    from concourse import bacc
    from concourse.tile import TileContext
    from concourse.masks import make_identity

    f32 = mybir.dt.float32
    fp16 = mybir.dt.float16
    Exp = mybir.ActivationFunctionType.Exp

    nc = bacc.Bacc("TRN2")

    hqT = nc.declare_dram_parameter("hqT", [D, R], fp16, isOutput=False)
    hkvT = nc.declare_dram_parameter("hkvT", [D, R], fp16, isOutput=False)
    wq = nc.declare_dram_parameter("wq", [D, 512], fp16, isOutput=False)
    wk = nc.declare_dram_parameter("wk", [D, 512], fp16, isOutput=False)
    wv = nc.declare_dram_parameter("wv", [D, 512], fp16, isOutput=False)
    wo = nc.declare_dram_parameter("wo", [512, D], fp16, isOutput=False)
    biasT = nc.declare_dram_parameter("biasT", [B, HPC, S, S], fp16, isOutput=False)
    outT = nc.declare_dram_parameter("outT", [D, R], fp16, isOutput=True)

    hq3 = hqT.rearrange("(t p) r -> p t r", p=128)  # [128, 32, 2048]
    hkv3 = hkvT.rearrange("(t p) r -> p t r", p=128)
    wq3 = wq.rearrange("(t p) m -> p t m", p=128)  # [128, 32, 512]
    wk3 = wk.rearrange("(t p) m -> p t m", p=128)
    wv3 = wv.rearrange("(t p) m -> p t m", p=128)
    wo3 = wo.rearrange("(t p) m -> p t m", p=128)  # [128, 4, 4096]
    biasT5 = biasT.rearrange("b h (t p) q -> b h p t q", p=128)  # [b,h,128,8,1024]
    outT3 = outT.rearrange("(m p) r -> p m r", p=128)  # [128, 32, 2048]

    with TileContext(nc) as tc:
      for _rep in range(repeat):
        with (
            tc.tile_pool(name="persist", bufs=1) as pers,
            tc.tile_pool(name="small", bufs=3) as spool,
            tc.tile_pool(name="wqpool", bufs=1) as wqp,
        ):
            qT_s = pers.tile([128, HPC, R], fp16)  # 16KB/part
            kT_s = pers.tile([128, HPC, R], fp16)  # 16KB/part
            v_s = pers.tile([128, 16, HPC, VW], fp16)  # 16.5KB/part
            ident = pers.tile([128, 128], fp16)
            make_identity(nc, ident)
            nc.vector.memset(v_s[:, :, :, 128:129], 1.0)  # softmax-denom ones

            # ---------------- fused k + v projection ----------------
            # kT[m, r] += Wk[kt, m].T @ hkvT[kt, r]   (4 psum banks)
            # v[r, c]  += hkvT[kt, r].T @ Wv[kt, c]   (4 psum banks)
            _hq_cm = tc.tile_pool(name="hq", bufs=8)
            hqp = _hq_cm.__enter__()
            hq_pre = {}
            with (
                tc.tile_pool(name="wkvpool", bufs=1) as wkvp,
                tc.tile_pool(name="hkv", bufs=8) as hkvp,
                tc.tile_pool(name="kvpsum", bufs=1, space="PSUM") as kvps,
            ):
                wk_chunks, wv_chunks = [], []
                for kt0, nkt, eng, which in (
                    (0, 2, nc.sync, "k"), (0, 2, nc.sync, "v"),
                    (2, 6, nc.gpsimd, "k"), (2, 6, nc.gpsimd, "v"),
                    (8, 8, nc.gpsimd, "k"), (8, 8, nc.gpsimd, "v"),
                    (16, 8, nc.gpsimd, "k"), (16, 8, nc.gpsimd, "v"),
                    (24, 8, nc.gpsimd, "k"), (24, 8, nc.gpsimd, "v"),
                ):
                    wsrc = wk3 if which == "k" else wv3
                    wt = wkvp.tile([128, nkt, 512], fp16,
                                   tag=f"w{which}{kt0}", name=f"w{which}{kt0}")
                    eng.dma_start(out=wt, in_=wsrc[:, kt0 : kt0 + nkt, :])
                    (wk_chunks if which == "k" else wv_chunks).append(
                        (wt, kt0, nkt))

                def w_at(chunks, kt):
                    for wt, kt0, nkt in chunks:
                        if kt0 <= kt < kt0 + nkt:
                            return wt[:, kt - kt0, :]
                    raise AssertionError

                # q weights prefetched behind the k/v chunks on gpsimd
                wq_chunks = []
                for kt0 in range(0, KT, 8):
                    wt = wqp.tile([128, 8, 512], fp16, tag=f"wq{kt0}",
                                  name=f"wq{kt0}")
                    nc.gpsimd.dma_start(out=wt, in_=wq3[:, kt0 : kt0 + 8, :])
                    wq_chunks.append((wt, kt0, 8))
                # first hq tiles for the q phase, also behind on gpsimd
                for ktg in range(6):
                    ht = hqp.tile([128, 2, 512], fp16, tag="ht", name="ht")
                    nc.gpsimd.dma_start(
                        out=ht, in_=hq3[:, ktg * 2 : ktg * 2 + 2, 0:512])
                    hq_pre[ktg] = ht

                for n in range(NB):
                    kpsums = [
                        kvps.tile([128, 512], f32, tag=f"kp{m}", name=f"kp{m}")
                        for m in range(4)
                    ]
                    vpsums = [
                        kvps.tile([128, 512], f32, tag=f"vp{j}", name=f"vp{j}")
                        for j in range(4)
                    ]
                    for ktg in range(16):
                        ht = hkvp.tile([128, 2, 512], fp16, tag="ht", name="ht")
                        (nc.scalar if ktg % 2 == 0 else nc.sync).dma_start(
                            out=ht,
                            in_=hkv3[:, ktg * 2 : ktg * 2 + 2,
                                     n * 512 : (n + 1) * 512],
                        )
                        for kl in range(2):
                            kt = ktg * 2 + kl
                            wkt = w_at(wk_chunks, kt)
                            wvt = w_at(wv_chunks, kt)
                            for m in range(4):
                                nc.tensor.matmul(
                                    kpsums[m],
                                    wkt[:, m * 128 : (m + 1) * 128],
                                    ht[:, kl, :],
                                    start=(kt == 0),
                                    stop=(kt == KT - 1),
                                )
                            for j in range(4):
                                nc.tensor.matmul(
                                    vpsums[j],
                                    ht[:, kl, j * 128 : (j + 1) * 128],
                                    wvt,
                                    start=(kt == 0),
                                    stop=(kt == KT - 1),
                                )
                    for m in range(4):
                        eng = nc.scalar if m % 2 == 0 else nc.vector
                        (eng.copy if m % 2 == 0 else eng.tensor_copy)(
                            out=kT_s[:, m, n * 512 : (n + 1) * 512], in_=kpsums[m]
                        )
                    for j in range(4):
                        eng = nc.scalar if j % 2 == 0 else nc.vector
                        (eng.copy if j % 2 == 0 else eng.tensor_copy)(
                            out=v_s[:, n * 4 + j, :, 0:128],
                            in_=vpsums[j].rearrange("p (h c) -> p h c", h=HPC),
                        )

            # ---------------- q projection + attention scope ----------------
            _wop_cm = tc.tile_pool(name="wopool", bufs=1)
            _bp_cm = tc.tile_pool(name="bias", bufs=12)
            wop = _wop_cm.__enter__()
            bpool = _bp_cm.__enter__()
            bias_tiles = {}

            def prefetch_bias(n, h, eng):
                b, qb = divmod(n, 2)
                tiles = []
                for g in range(4):
                    bias_t = bpool.tile([128, 2, 512], fp16, tag="bias",
                                        name="bias_t")
                    eng.dma_start(
                        out=bias_t,
                        in_=biasT5[b, h][:, g * 2 : (g + 1) * 2,
                                         qb * 512 : (qb + 1) * 512],
                    )
                    tiles.append(bias_t)
                bias_tiles[(n, h)] = tiles

            # qT[m, r] += Wq[kt, m].T @ hqT[kt, r]; SCALE folded into Wq on host
            with (
                tc.tile_pool(name="qpsum", bufs=2, space="PSUM") as qps,
            ):
                # warm the Exp activation table (~2.7us) well before attention;
                # doing this at t=0 would block the scalar DMA ring.
                dummy = spool.tile([128, 1], f32, tag="dummy", name="dummy")
                nc.vector.memset(dummy, 0.0)
                nc.scalar.activation(out=dummy, in_=dummy, func=Exp)
                # prefetches for the attention phase (bandwidth is slack here)
                prefetch_bias(0, 0, nc.gpsimd)
                prefetch_bias(0, 1, nc.gpsimd)
                wo_s = wop.tile([128, HPC, D], fp16)  # 32KB/part
                nc.gpsimd.dma_start(out=wo_s, in_=wo3)

                def wq_at(kt):
                    for wt, kt0, nkt in wq_chunks:
                        if kt0 <= kt < kt0 + nkt:
                            return wt[:, kt - kt0, :]
                    raise AssertionError

                for n in range(NB):
                    psums = [
                        qps.tile([128, 512], f32, tag=f"qp{m}", name=f"qp{m}")
                        for m in range(4)
                    ]
                    for ktg in range(16):  # groups of 2 kt
                        if n == 0 and ktg < 6:
                            ht = hq_pre[ktg]
                        else:
                            ht = hqp.tile([128, 2, 512], fp16, tag="ht",
                                          name="ht")
                            (nc.scalar if ktg % 2 == 0 else nc.sync).dma_start(
                                out=ht,
                                in_=hq3[:, ktg * 2 : ktg * 2 + 2,
                                        n * 512 : (n + 1) * 512],
                            )
                        for kl in range(2):
                            kt = ktg * 2 + kl
                            for m in range(4):
                                nc.tensor.matmul(
                                    psums[m],
                                    wq_at(kt)[:, m * 128 : (m + 1) * 128],
                                    ht[:, kl, :],
                                    start=(kt == 0),
                                    stop=(kt == KT - 1),
                                )
                    for m in range(4):
                        eng = nc.scalar if m % 2 == 0 else nc.vector
                        (eng.copy if m % 2 == 0 else eng.tensor_copy)(
                            out=qT_s[:, m, n * 512 : (n + 1) * 512], in_=psums[m]
                        )

            # ---------------- attention + output projection ----------------
            with (
                tc.tile_pool(name="probs", bufs=2) as ppool,
                tc.tile_pool(name="ctxT", bufs=2) as ctpool,
                tc.tile_pool(name="obuf", bufs=4) as opool,
                tc.tile_pool(name="csb", bufs=4) as csbp,
                tc.tile_pool(name="spsum", bufs=2, space="PSUM") as sps,
                tc.tile_pool(name="cpsum", bufs=1, space="PSUM") as cps,
                tc.tile_pool(name="tpsum", bufs=1, space="PSUM") as tps,
                tc.tile_pool(name="opsum", bufs=2, space="PSUM") as ops,
            ):
                probs_of = {}
                ctxT_of = {}
                hseq = [(n, h) for n in range(NB) for h in range(HPC)]

                def scores(n, h):
                    """scoresT[k, q] = bias + K.Q^T; exp -> probsU [k, 8, 512]."""
                    b, qb = divmod(n, 2)
                    # prefetch bias two heads ahead on the now-idle sync queue
                    i = hseq.index((n, h))
                    if i + 2 < len(hseq):
                        prefetch_bias(*hseq[i + 2], nc.sync)
                    probsU = ppool.tile([128, 8, 512], fp16, tag="probs",
                                        name="probsU")
                    probs_of[(n, h)] = probsU
                    btiles = bias_tiles.pop((n, h))
                    for g in range(4):  # 2 k-tiles per group
                        s_ps = sps.tile([128, 2, 512], f32, tag="s", name="s_ps")
                        bias_t = btiles[g]
                        pe_bias = True  # DVE-add variant stalls the exp chain
                        for j in range(2):
                            kt = g * 2 + j
                            if pe_bias:
                                nc.tensor.matmul(
                                    s_ps[:, j, :], ident, bias_t[:, j, :],
                                    start=True, stop=False,
                                )
                            nc.tensor.matmul(
                                s_ps[:, j, :],
                                kT_s[:, h, b * 1024 + kt * 128 : b * 1024 + (kt + 1) * 128],
                                qT_s[:, h, n * 512 : (n + 1) * 512],
                                start=not pe_bias, stop=True,
                            )
                        if not pe_bias:
                            nc.vector.tensor_add(out=s_ps, in0=s_ps, in1=bias_t)
                        nc.scalar.activation(
                            out=probsU[:, g * 2 : (g + 1) * 2, :], in_=s_ps,
                            func=Exp,
                        )

                def ctx(n, h):
                    """ctx[q, dh+1] = probsU.T @ [v | 1]; normalize; transpose."""
                    b = n // 2
                    probsU = probs_of.pop((n, h))
                    t_ps = tps.tile([128, 4, 128], fp16, tag="t", name="t_ps")
                    for pair in range(2):
                        c_ps = cps.tile([128, 2, 130], f32, tag="c", name="c_ps")
                        for qt2 in range(2):
                            qt = pair * 2 + qt2
                            for kt in range(8):
                                nc.tensor.matmul(
                                    c_ps[:, qt2, 0:129],
                                    probsU[:, kt, qt * 128 : (qt + 1) * 128],
                                    v_s[:, b * 8 + kt, h, 0:129],
                                    start=(kt == 0),
                                    stop=(kt == 7),
                                )
                        for qt2 in range(2):
                            qt = pair * 2 + qt2
                            recip = spool.tile([128, 1], f32, tag="recip",
                                               name="recip")
                            nc.vector.reciprocal(out=recip,
                                                 in_=c_ps[:, qt2, 128:129])
                            ctx_sb = csbp.tile([128, 128], fp16, tag="ctxsb",
                                               name="ctx_sb")
                            nc.vector.tensor_scalar_mul(
                                out=ctx_sb, in0=c_ps[:, qt2, 0:128],
                                scalar1=recip,
                            )
                            nc.tensor.transpose(t_ps[:, qt, :], ctx_sb, ident)
                    ctxT = ctxT_of[n]
                    nc.vector.tensor_copy(
                        out=ctxT[:, h, :],
                        in_=t_ps.rearrange("p a q -> p (a q)"),
                    )

                def outproj(n):
                    ctxT = ctxT_of.pop(n)
                    for m in range(KT):
                        o_ps = ops.tile([128, 512], f32, tag="o", name="o_ps")
                        for t in range(HPC):
                            nc.tensor.matmul(
                                o_ps,
                                wo_s[:, t, m * 128 : (m + 1) * 128],
                                ctxT[:, t, :],
                                start=(t == 0),
                                stop=(t == HPC - 1),
                            )
                        osb = opool.tile([128, 512], fp16, tag="osb", name="osb")
                        if m % 2 == 0:
                            nc.scalar.copy(out=osb, in_=o_ps)
                        else:
                            nc.vector.tensor_copy(out=osb, in_=o_ps)
                        if n == NB - 1:
                            dmae = (nc.gpsimd, nc.scalar, nc.sync)[m % 3]
                        else:
                            dmae = (nc.gpsimd, nc.scalar)[m % 2]
                        dmae.dma_start(
                            out=outT3[:, m, n * 512 : (n + 1) * 512], in_=osb
                        )

                # software-pipelined emission: scores one head ahead of ctx;
                # scores(n+1, 0) before outproj(n) so exp overlaps the
                # output projection.
                for n in range(NB):
                    ctxT_of[n] = ctpool.tile([128, HPC, 512], fp16, tag="ctxT",
                                             name="ctxT")
                    if n == 0:
                        scores(0, 0)
                    for h in range(HPC):
                        if h < HPC - 1:
                            scores(n, h + 1)
                        ctx(n, h)
                    if n < NB - 1:
                        scores(n + 1, 0)
                    outproj(n)

            _bp_cm.__exit__(None, None, None)
            _wop_cm.__exit__(None, None, None)
            _hq_cm.__exit__(None, None, None)

    nc.compile()
    return nc


_NC_CACHE = None


def _prep_in_maps(
    hidden_q, hidden_kv, attention_mask, position_bias, Wq, Wk, Wv, Wo
):
    hqT = np.ascontiguousarray(
        np.asarray(hidden_q, dtype=np.float32).reshape(R, D).T
    ).astype(np.float16)
    hkvT = np.ascontiguousarray(
        np.asarray(hidden_kv, dtype=np.float32).reshape(R, D).T
    ).astype(np.float16)
    mask = np.asarray(attention_mask)  # [B, S, S] (q, k)
    pb = np.asarray(position_bias, dtype=np.float32)
    Wq = np.asarray(Wq, dtype=np.float32) * np.float32(SCALE)

    in_maps = []
    for c in range(NCORES):
        h0 = c * HPC
        bias_c = np.where(
            mask[:, None, :, :], pb[:, h0 : h0 + HPC], np.float32(MASK_NEG)
        )
        # transpose to [b, h, k, q]
        biasT_c = np.ascontiguousarray(bias_c.transpose(0, 1, 3, 2)).astype(
            np.float16
        )
        in_maps.append(
            {
                "hqT": hqT,
                "hkvT": hkvT,
                "wq": np.ascontiguousarray(
                    Wq[:, h0 * DH : (h0 + HPC) * DH]
                ).astype(np.float16),
                "wk": np.ascontiguousarray(
                    Wk[:, h0 * DH : (h0 + HPC) * DH]
                ).astype(np.float16),
                "wv": np.ascontiguousarray(
                    Wv[:, h0 * DH : (h0 + HPC) * DH]
                ).astype(np.float16),
                "wo": np.ascontiguousarray(
                    Wo[h0 * DH : (h0 + HPC) * DH, :]
                ).astype(np.float16),
                "biasT": biasT_c,
            }
        )
    return in_maps


def kernel(
    hidden_q: np.ndarray,
    hidden_kv: np.ndarray,
    attention_mask: np.ndarray,
    position_bias: np.ndarray,
    Wq: np.ndarray,
    Wk: np.ndarray,
    Wv: np.ndarray,
    Wo: np.ndarray,
) -> np.ndarray:
    from concourse.bass_utils import run_bass_kernel_spmd

    global _NC_CACHE
    if _NC_CACHE is None:
        _NC_CACHE = _build_core_kernel()
    nc = _NC_CACHE

    in_maps = _prep_in_maps(
        hidden_q, hidden_kv, attention_mask, position_bias, Wq, Wk, Wv, Wo
    )
    res = run_bass_kernel_spmd(nc, in_maps, list(range(NCORES)))
    acc = res.results[0]["outT"].astype(np.float32)
    for c in range(1, NCORES):
        acc += res.results[c]["outT"]
    return np.ascontiguousarray(acc.T).reshape(B, S, D)
